# revision 16
# baseline (speedup 1.0000x reference)
"""Causal multi-head self-attention on 8 TRN2 NeuronCores.

Problem (hardcoded): x [4, 2048, 1024] f32, qkv_w [1024, 3072], proj_w
[1024, 1024], proj_b [1024], 16 heads of dim 64, causal softmax.

Sharding: core c handles batch b = c // 2 and head-half c % 2 (8 of the 16
heads). Each core computes the QKV projection for its 8 heads, causal
attention, and the partial output projection (its 512 rows of proj_w). The
host sums the two partials per batch and adds the bias.

On-core dataflow (head-dim on partitions everywhere):
  xT    = transpose(x) via PE-transpose                      [D, N]
  qT/kT = W.T @ x.T  (f32r matmuls, stored bf16)             [128, N] per pair
  v     = x @ Wv     (f32r, stored bf16, k-rows on parts)    [N, 512]
  scoresT[k,q] per head = kT-slice.T @ qT   (bf16, row-packed head pairs)
  expT  = exp(0.125 * scoresT) on ACT, tri-mask on the diagonal 128-block
  outT_unnorm[dh,q] += v-slice.T @ expT     (accumulated over k chunks)
  sums via ones.T @ expT matmuls -> reciprocal -> sel-matrix outer product
  outT  = outT_unnorm * recip-broadcast; partial = outT.T @ proj_w (f32r)
"""

import numpy as np

P = 128
N = 2048
D = 1024
DH = 512          # head dims per core (8 heads x 64)
HD = 64
HP = HD + 1       # head dims + ones column (softmax denominator row)
DHP = 8 * HP      # per-row-chunk v columns incl. ones (520)
NPAIR = 4
DC = D // P       # 8 contraction chunks
NRC = N // P      # 16 row chunks
NQC = N // 512    # 4 query 512-chunks

_CACHE = {}


def _build_nc(reps=1):
    from contextlib import ExitStack

    import concourse.bacc as bacc
    import concourse.tile as tile
    from concourse import mybir

    f32 = mybir.dt.float32
    f32r = mybir.dt.float32r
    bf16 = mybir.dt.bfloat16
    AF = mybir.ActivationFunctionType

    nc = bacc.Bacc("TRN2", target_bir_lowering=False, debug=False,
                   enable_asserts=False, num_devices=8)

    xt = nc.dram_tensor("xt", [D, N], f32r, kind="ExternalInput").ap()
    wq = nc.dram_tensor("wq", [D, DH], f32r, kind="ExternalInput").ap()
    wk = nc.dram_tensor("wk", [D, DH], f32r, kind="ExternalInput").ap()
    wv = nc.dram_tensor("wv", [D, DH], f32r, kind="ExternalInput").ap()
    pw = nc.dram_tensor("pw", [DH, D], f32r, kind="ExternalInput").ap()
    tri = nc.dram_tensor("tri", [P, P], f32, kind="ExternalInput").ap()
    sel = nc.dram_tensor("sel", [P, P], f32, kind="ExternalInput").ap()
    out = nc.dram_tensor("out", [N, D], f32, kind="ExternalOutput").ap()

    def emit_rep(tc, const_tiles):
        tri_b, ones_b, sel_r, onesw_r = const_tiles
        with ExitStack() as rep:
            big_ps = rep.enter_context(
                tc.tile_pool(name="big_ps", bufs=2, space="PSUM"))
            small_ps = rep.enter_context(
                tc.tile_pool(name="small_ps", bufs=2, space="PSUM"))
            po_ps = rep.enter_context(
                tc.tile_pool(name="po_ps", bufs=1, space="PSUM"))
            outT_pool = rep.enter_context(tc.tile_pool(name="outT", bufs=4))
            outT = [outT_pool.tile([P, N], f32r, name=f"outT{p}", tag="outT")
                    for p in range(NPAIR)]

            pw_pool = rep.enter_context(tc.tile_pool(name="pw", bufs=1))
            osb = rep.enter_context(tc.tile_pool(name="osb", bufs=4))
            pw_sb = pw_pool.tile([P, NPAIR * D], f32r)

            with ExitStack() as mid:
                qkv = mid.enter_context(tc.tile_pool(name="qkv", bufs=16))
                xT_pool = mid.enter_context(tc.tile_pool(name="xT", bufs=8))
                xT = [xT_pool.tile([P, N], f32r, name=f"xT{dc}", tag="xT")
                      for dc in range(DC)]
                vt_pool = mid.enter_context(tc.tile_pool(name="vt", bufs=1))
                # per head: 64 v-columns + a ones column, so the AV matmul's
                # 65th output partition accumulates the softmax denominator
                v_sb = vt_pool.tile([P, NRC * DHP], bf16)
                wv_pool = mid.enter_context(tc.tile_pool(name="wv", bufs=8))

                # two HWDGE queues stream in parallel: weights on the SP
                # queue, xT column-chunks on the Act queue, so phase B2 can
                # start once wv + the first xT chunk land (~16 DMAs)
                wv_t = []
                for dc in range(DC):
                    t = wv_pool.tile([P, DH], f32r, name="wvt")
                    nc.sync.dma_start(t[:], wv[dc * P:(dc + 1) * P, :])
                    wv_t.append(t)
                w0q, w0k = [], []
                for dc in range(DC):
                    tq = qkv.tile([P, P], f32r, tag="wq")
                    nc.sync.dma_start(tq[:], wq[dc * P:(dc + 1) * P, 0:P])
                    w0q.append(tq)
                    tk = qkv.tile([P, P], f32r, tag="wk")
                    nc.sync.dma_start(tk[:], wk[dc * P:(dc + 1) * P, 0:P])
                    w0k.append(tk)
                for qw in range(4):
                    for dc in range(DC):
                        nc.scalar.dma_start(
                            xT[dc][:, qw * 512:(qw + 1) * 512],
                            xt[dc * P:(dc + 1) * P, qw * 512:(qw + 1) * 512])
                for pp in range(NPAIR):
                    nc.sync.dma_start(pw_sb[:, pp * D:(pp + 1) * D],
                                      pw[pp * P:(pp + 1) * P, :])

                # ---- Phase B2: v = x @ Wv (k-rows on partitions), bf16 ----
                for rc in range(NRC):
                    pv = small_ps.tile([P, DH], f32, tag="sp")
                    for dc in range(DC):
                        nc.tensor.matmul(
                            pv[:],
                            xT[dc][:, rc * P:(rc + 1) * P],
                            wv_t[dc][:], start=(dc == 0), stop=(dc == DC - 1))
                    dst = v_sb[:, rc * DHP:(rc + 1) * DHP].rearrange(
                        "p (h c) -> p h c", h=8)
                    nc.vector.tensor_copy(
                        dst[:, :, 0:HD],
                        pv[:].rearrange("p (h c) -> p h c", h=8))
                    nc.vector.memset(dst[:, :, HD:HP], 1.0)

                qkT = mid.enter_context(tc.tile_pool(name="qkT", bufs=2))
                expp = mid.enter_context(tc.tile_pool(name="expp", bufs=5))
                ssbp = mid.enter_context(tc.tile_pool(name="ssb", bufs=2))
                drc_pool = mid.enter_context(tc.tile_pool(name="drc", bufs=1))
                # two persistent divisor-staging tiles, ones-filled once; rows
                # 0/32 are rewritten by reciprocal each round, rows 1-31 stay 1
                ssb_t = []
                for i in range(2):
                    t2 = ssbp.tile([P, 512], f32r, name=f"ssbt{i}", tag="ssb")
                    nc.vector.tensor_copy(t2[0:HD, :], onesw_r[0:HD, :])
                    ssb_t.append(t2)

                wq_t, wk_t = w0q, w0k
                for p in range(NPAIR):
                    # ---- Phase B1: qT/kT for this head pair (f32r->bf16) --
                    qT = qkT.tile([P, N], bf16, tag="qT")
                    kT = qkT.tile([P, N], bf16, tag="kT")
                    for qc in range(NQC):
                        pqk = big_ps.tile([P, 1024], f32, tag="bp")
                        for dc in range(DC):
                            rhs = xT[dc][:, qc * 512:(qc + 1) * 512]
                            nc.tensor.matmul(pqk[:, 0:512], wq_t[dc][:], rhs,
                                             start=(dc == 0), stop=(dc == DC - 1))
                            nc.tensor.matmul(pqk[:, 512:1024], wk_t[dc][:], rhs,
                                             start=(dc == 0), stop=(dc == DC - 1))
                        nc.vector.tensor_copy(
                            qT[:, qc * 512:(qc + 1) * 512], pqk[:, 0:512])
                        nc.vector.tensor_copy(
                            kT[:, qc * 512:(qc + 1) * 512], pqk[:, 512:1024])
                    # prefetch next pair's weights now: the ring buffers were
                    # last read by the qkT matmuls above, so these DMAs flow
                    # during this pair's attention phase
                    if p + 1 < NPAIR:
                        wq_t, wk_t = [], []
                        for dc in range(DC):
                            tq = qkv.tile([P, P], f32r, tag="wq")
                            nc.sync.dma_start(
                                tq[:],
                                wq[dc * P:(dc + 1) * P,
                                   (p + 1) * P:(p + 2) * P])
                            wq_t.append(tq)
                            tk = qkv.tile([P, P], f32r, tag="wk")
                            nc.sync.dma_start(
                                tk[:],
                                wk[dc * P:(dc + 1) * P,
                                   (p + 1) * P:(p + 2) * P])
                            wk_t.append(tk)

                    # ---- Attention for this pair --------------------------
                    for qc4 in range(NQC):
                        nkc = 4 * qc4 + 4
                        po = [po_ps.tile([HP, 512], f32, name=f"po{e}",
                                         tag=f"po{e}") for e in range(2)]
                        for kc in range(nkc):
                            qoff = max(0, kc * P - qc4 * 512)
                            q0 = qc4 * 512 + qoff
                            q1 = (qc4 + 1) * 512
                            ps_s = big_ps.tile([P, 1024], f32, name="ps_s", tag="bp")
                            for e in range(2):
                                nc.tensor.matmul(
                                    ps_s[:, e * 512 + qoff: e * 512 + 512],
                                    kT[e * HD:(e + 1) * HD, kc * P:(kc + 1) * P],
                                    qT[e * HD:(e + 1) * HD, q0:q1],
                                    start=True, stop=True)
                            et = expp.tile([P, 1024], bf16, name="et", tag="et")
                            ev = et[:].rearrange("p (h q) -> p h q", h=2)[:, :, qoff:512]
                            pv_ = ps_s[:].rearrange("p (h q) -> p h q", h=2)[:, :, qoff:512]
                            nc.scalar.activation(ev, pv_, AF.Exp, scale=0.125)
                            if kc >= 4 * qc4:  # diagonal block -> causal mask
                                em = et[:].rearrange("p (h q) -> p h q", h=2)[
                                    :, :, qoff:qoff + P]
                                trib = tri_b[:].rearrange("p (a q) -> p a q", a=1)\
                                    .broadcast_to([P, 2, P])
                                nc.vector.tensor_mul(em, em, trib)
                            for e in range(2):
                                h = 2 * p + e
                                nc.tensor.matmul(
                                    po[e][0:HP, qoff:512],
                                    v_sb[:, kc * DHP + h * HP:
                                         kc * DHP + (h + 1) * HP],
                                    et[:, e * 512 + qoff: e * 512 + 512],
                                    start=(kc == 0), stop=(kc == nkc - 1),
                                    skip_group_check=True)
                        # stage raw sums (rounded to f32r), broadcast to all
                        # 128 partitions via sel-matmul, then one partition-
                        # parallel fast reciprocal over the broadcast tile
                        ssb = ssb_t[(p * NQC + qc4) % 2]
                        with nc.allow_low_precision(reason="f32r divisor rows"):
                            nc.vector.tensor_copy(ssb[0:1, :], po[0][HD:HP, :])
                            nc.vector.tensor_copy(ssb[32:33, :], po[1][HD:HP, :])
                        dps = small_ps.tile([P, 512], f32, name="dps", tag="sp")
                        nc.tensor.matmul(dps[:], sel_r[0:33, :], ssb[0:33, :],
                                         start=True, stop=True)
                        drc = drc_pool.tile([P, 512], f32, tag="drc")
                        nc.vector.reciprocal_approx_fast(drc[:], dps[:])
                        oslice = outT[p][:, qc4 * 512:(qc4 + 1) * 512]
                        nc.vector.tensor_copy(
                            outT[p][0:HD, qc4 * 512:(qc4 + 1) * 512],
                            po[0][0:HD, :])
                        nc.scalar.copy(
                            outT[p][HD:P, qc4 * 512:(qc4 + 1) * 512],
                            po[1][0:HD, :])
                        nc.vector.tensor_mul(oslice, oslice, drc[:])

                        # ---- Phase D: output projection, interleaved into
                        # the last pair's attention (per completed q-window)
                        if p == NPAIR - 1:
                            for rc in range(4 * qc4, 4 * qc4 + 4):
                                for cc in range(2):
                                    pr = small_ps.tile([P, 512], f32,
                                                       name="pr", tag="sp")
                                    for pp in range(NPAIR):
                                        nc.tensor.matmul(
                                            pr[:],
                                            outT[pp][:, rc * P:(rc + 1) * P],
                                            pw_sb[:, pp * D + cc * 512:
                                                  pp * D + (cc + 1) * 512],
                                            start=(pp == 0),
                                            stop=(pp == NPAIR - 1))
                                    ot = osb.tile([P, 512], f32,
                                                  name="ot", tag="osb")
                                    nc.scalar.copy(ot[:], pr[:])
                                    nc.sync.dma_start(
                                        out[rc * P:(rc + 1) * P,
                                            cc * 512:(cc + 1) * 512], ot[:])

    with tile.TileContext(nc) as tc, ExitStack() as ctx:
        const = ctx.enter_context(tc.tile_pool(name="const", bufs=1))
        tri_f = const.tile([P, P], f32)
        nc.sync.dma_start(tri_f[:], tri)
        tri_b = const.tile([P, P], bf16)
        nc.vector.tensor_copy(tri_b[:], tri_f[:])
        ones_b = const.tile([P, 1], bf16)
        nc.vector.memset(ones_b[:], 1.0)
        sel_f = const.tile([P, P], f32)
        nc.sync.dma_start(sel_f[:], sel)
        sel_r = const.tile([P, P], f32r)
        nc.vector.tensor_copy(sel_r[:], sel_f[:])
        onesw_f = const.tile([P, 512], f32)
        nc.vector.memset(onesw_f[:], 1.0)
        onesw_r = const.tile([P, 512], f32r)
        nc.vector.tensor_copy(onesw_r[:], onesw_f[:])
        const_tiles = (tri_b, ones_b, sel_r, onesw_r)
        for _rep in range(reps):
            emit_rep(tc, const_tiles)

    nc.compile()
    return nc


def get_nc(reps=1):
    key = f"nc{reps}"
    if key not in _CACHE:
        _CACHE[key] = _build_nc(reps=reps)
    return _CACHE[key]


def _make_runner(nc, n_cores=8):
    """Cached jit over the bass_exec primitive (mirrors
    bass2jax.run_bass_via_pjrt's multi-core path, but reusable across calls
    so jax does not re-trace per invocation)."""
    import jax
    from jax.sharding import Mesh, PartitionSpec
    from jax.experimental.shard_map import shard_map
    from concourse import bass2jax, mybir

    bass2jax.install_neuronx_cc_hook()
    part_name = nc.partition_id_tensor.name if nc.partition_id_tensor else None
    in_names, out_names, out_avals, zero_templates = [], [], [], []
    for alloc in nc.m.functions[0].allocations:
        if not isinstance(alloc, mybir.MemoryLocationSet):
            continue
        name = alloc.memorylocations[0].name
        if alloc.kind == "ExternalInput":
            if name != part_name:
                in_names.append(name)
        elif alloc.kind == "ExternalOutput":
            out_names.append(name)
            shape = tuple(alloc.tensor_shape)
            dtype = mybir.dt.np(alloc.dtype)
            out_avals.append(jax.core.ShapedArray(shape, dtype))
            zero_templates.append((shape, dtype))
    n_params = len(in_names)
    n_outs = len(out_avals)
    all_names = in_names + out_names + ([part_name] if part_name else [])

    def _body(*args):
        operands = list(args)
        if part_name:
            operands.append(bass2jax.partition_id_tensor())
        outs = bass2jax._bass_exec_p.bind(
            *operands,
            out_avals=tuple(out_avals),
            in_names=tuple(all_names),
            out_names=tuple(out_names),
            lowering_input_output_aliases=(),
            sim_require_finite=True,
            sim_require_nnan=True,
            nc=nc,
        )
        return tuple(outs)

    devices = jax.devices()[:n_cores]
    mesh = Mesh(np.asarray(devices), ("core",))
    in_specs = (PartitionSpec("core"),) * (n_params + n_outs)
    out_specs = (PartitionSpec("core"),) * n_outs
    donate = tuple(range(n_params, n_params + n_outs))
    sharded = jax.jit(
        shard_map(_body, mesh=mesh, in_specs=in_specs, out_specs=out_specs,
                  check_rep=False),
        donate_argnums=donate, keep_unused=True)

    def run(in_maps):
        concat_in = [
            np.concatenate([np.asarray(m[name]) for m in in_maps], axis=0)
            for name in in_names
        ]
        concat_zeros = [
            np.zeros((n_cores * s[0], *s[1:]), d) for s, d in zero_templates
        ]
        out_arrs = sharded(*concat_in, *concat_zeros)
        return {
            name: np.asarray(out_arrs[i]).reshape(n_cores, *zero_templates[i][0])
            for i, name in enumerate(out_names)
        }

    run.sharded = sharded
    run.mesh = mesh
    run.in_names = in_names
    run.out_names = out_names
    run.zero_templates = zero_templates
    run.n_cores = n_cores
    return run


def get_runner(reps=1):
    key = f"runner{reps}"
    if key not in _CACHE:
        _CACHE[key] = _make_runner(get_nc(reps=reps))
    return _CACHE[key]


def make_in_maps(x, qkv_w, proj_w):
    x = np.asarray(x, dtype=np.float32)
    qkv_w = np.asarray(qkv_w, dtype=np.float32)
    proj_w = np.asarray(proj_w, dtype=np.float32)
    tri = np.triu(np.ones((P, P), dtype=np.float32))
    sel = np.zeros((P, P), dtype=np.float32)
    sel[0, 0:64] = 1.0
    sel[32, 64:128] = 1.0
    in_maps = []
    for c in range(8):
        b, half = c // 2, c % 2
        hs = half * DH
        in_maps.append({
            "xt": np.ascontiguousarray(x[b].T),
            "wq": np.ascontiguousarray(qkv_w[:, hs:hs + DH]),
            "wk": np.ascontiguousarray(qkv_w[:, D + hs:D + hs + DH]),
            "wv": np.ascontiguousarray(qkv_w[:, 2 * D + hs:2 * D + hs + DH]),
            "pw": np.ascontiguousarray(proj_w[hs:hs + DH, :]),
            "tri": tri,
            "sel": sel,
        })
    return in_maps


def kernel(x, qkv_w, proj_w, proj_b, **_):
    proj_b = np.asarray(proj_b, dtype=np.float32)
    run = get_runner()
    in_maps = make_in_maps(x, qkv_w, proj_w)
    parts = run(in_maps)["out"]
    outp = np.empty((4, N, D), dtype=np.float32)
    for b in range(4):
        outp[b] = parts[2 * b] + parts[2 * b + 1] + proj_b[None, :]
    return outp

